# revision 1
# baseline (speedup 1.0000x reference)
"""Trainium2 Bass kernel for nn_MoELayer (dense MoE: gate softmax over 8
experts, all experts computed, gate-weighted sum).

Strategy: data-parallel over tokens. B*S = 8192 tokens are split across the
8 NeuronCores (1024 tokens each); every core holds all expert weights
(replicated) and computes its token slice end-to-end, so no collective is
needed and per-core outputs are disjoint slices of the final [B,S,H] output.

Device kernel (per core, SPMD):
  - x is staged host-side as x_T_aug [1152, 1024] bf16: rows 0..1023 are
    x.T (contraction dim on partitions), row 1024 is ones, rest zeros.
    Weights are staged as We_T_aug [8, 1152, 2048] with the bias be in row
    1024 — the bias add is folded into the matmul contraction (K = 9x128).
  - gate: logits via TensorE (N=8), softmax on DVE/ACT (reduce_max(negate),
    Exp(bias=-max), reduce_add, reciprocal, scale).
  - experts: for each expert, [128-token tile] x [512-wide H chunk] bf16
    matmuls accumulate K=9 chunks into a 4-bank PSUM tile [128, 2048].
    Epilogue fuses leaky-relu and gate weighting into ONE ScalarE op:
    Lrelu(g * x) = g * Lrelu(x) for g > 0 (softmax probs are positive),
    then a single VectorE add accumulates over experts.
"""

import numpy as np
import ml_dtypes

BF16 = ml_dtypes.bfloat16

B, S, D, H, E = 4, 2048, 1024, 2048, 8
NCORES = 8
TOK = B * S                 # 8192 tokens
TPC = TOK // NCORES         # 1024 tokens per core
P = 128
KCH = (D // P) + 1          # 9 contraction chunks (8 data + 1 bias/zeros)
KAUG = KCH * P              # 1152
NTT = TPC // P              # 8 token tiles per core
NHC = H // 512              # 4 H chunks

_CACHE = {}


def _build_nc():
    import concourse.mybir as mybir
    import concourse.tile as tile
    from concourse import bacc
    from concourse.bass import ts, ds

    fp32 = mybir.dt.float32
    bf16 = mybir.dt.bfloat16
    AF = mybir.ActivationFunctionType
    Alu = mybir.AluOpType

    nc = bacc.Bacc("TRN2", target_bir_lowering=False, debug=False)

    xT_d = nc.dram_tensor("xT", [KAUG, TPC], bf16, kind="ExternalInput")
    weT_d = nc.dram_tensor("weT", [E, KAUG, H], bf16, kind="ExternalInput")
    wgT_d = nc.dram_tensor("wgT", [KAUG, E], bf16, kind="ExternalInput")
    out_d = nc.dram_tensor("out", [TPC, H], fp32, kind="ExternalOutput")

    with tile.TileContext(nc) as tc:
        with (
            tc.tile_pool(name="const", bufs=1) as const_pool,
            tc.tile_pool(name="wep", bufs=2) as we_pool,
            tc.tile_pool(name="accp", bufs=1) as acc_pool,
            tc.tile_pool(name="leakp", bufs=3) as leak_pool,
            tc.tile_pool(name="smallp", bufs=8) as small_pool,
        ):
            x_sb = const_pool.tile([P, KCH, TPC], bf16)
            nc.sync.dma_start(x_sb[:], xT_d.ap().rearrange("(c p) t -> p c t", p=P))
            wg_sb = const_pool.tile([P, KCH, E], bf16)
            nc.sync.dma_start(wg_sb[:], wgT_d.ap().rearrange("(c p) e -> p c e", p=P))

            g_all = const_pool.tile([P, NTT, E], fp32)
            acc = acc_pool.tile([P, NTT, H], fp32)

            # ---------------- gate phase ----------------
            with tc.tile_pool(name="gps", bufs=2, space="PSUM") as gps_pool:
                for tt in range(NTT):
                    gl = gps_pool.tile([P, E], fp32, tag="gl")
                    for kc in range(KCH):
                        nc.tensor.matmul(gl, x_sb[:, kc, ts(tt, P)],
                                         wg_sb[:, kc, :],
                                         start=(kc == 0), stop=(kc == KCH - 1))
                    negmax = small_pool.tile([P, 1], fp32, tag="negmax")
                    nc.vector.tensor_reduce(negmax, gl, axis=mybir.AxisListType.X,
                                            op=Alu.max, negate=True)
                    expd = small_pool.tile([P, E], fp32, tag="expd")
                    nc.scalar.activation(expd, gl, AF.Exp, bias=negmax, scale=1.0)
                    ssum = small_pool.tile([P, 1], fp32, tag="ssum")
                    nc.vector.tensor_reduce(ssum, expd, axis=mybir.AxisListType.X,
                                            op=Alu.add)
                    rec = small_pool.tile([P, 1], fp32, tag="rec")
                    nc.vector.reciprocal(rec, ssum)
                    nc.vector.tensor_scalar_mul(g_all[:, tt, :], expd, rec)

            # ---------------- expert phase ----------------
            with tc.tile_pool(name="mmps", bufs=2, space="PSUM") as mm_pool:
                for e in range(E):
                    we_sb = we_pool.tile([P, KCH, H], bf16, tag="we")
                    nc.sync.dma_start(
                        we_sb[:], weT_d.ap()[e].rearrange("(c p) h -> p c h", p=P))
                    for tt in range(NTT):
                        ps = mm_pool.tile([P, H], fp32, tag="ps")
                        for hc in range(NHC):
                            for kc in range(KCH):
                                nc.tensor.matmul(
                                    ps[:, ds(hc * 512, 512)],
                                    x_sb[:, kc, ts(tt, P)],
                                    we_sb[:, kc, ds(hc * 512, 512)],
                                    start=(kc == 0), stop=(kc == KCH - 1))
                        gap = g_all[:, tt, ds(e, 1)]
                        if e == 0:
                            nc.scalar.activation(acc[:, tt, :], ps, AF.Lrelu,
                                                 scale=gap, alpha=0.01)
                        else:
                            leak = leak_pool.tile([P, H], fp32, tag="leak")
                            nc.scalar.activation(leak, ps, AF.Lrelu,
                                                 scale=gap, alpha=0.01)
                            nc.vector.tensor_add(acc[:, tt, :], acc[:, tt, :], leak)

            for tt in range(NTT):
                nc.sync.dma_start(out_d.ap()[ts(tt, P), :], acc[:, tt, :])

    nc.compile()
    return nc


def _get_nc():
    if "nc" not in _CACHE:
        _CACHE["nc"] = _build_nc()
    return _CACHE["nc"]


def _prep_host(inputs, Wg, bg, We, be):
    inputs = np.asarray(inputs, dtype=np.float32)
    Wg = np.asarray(Wg, dtype=np.float32)
    bg = np.asarray(bg, dtype=np.float32)
    We = np.asarray(We, dtype=np.float32)
    be = np.asarray(be, dtype=np.float32)

    xT = np.zeros((KAUG, TOK), BF16)
    xT[:D] = np.ascontiguousarray(inputs.reshape(TOK, D).T).astype(BF16)
    xT[D] = np.asarray(1.0, BF16)

    weT = np.zeros((E, KAUG, H), BF16)
    weT[:, :D] = np.ascontiguousarray(We.transpose(0, 2, 1)).astype(BF16)
    weT[:, D] = be.astype(BF16)

    wgT = np.zeros((KAUG, E), BF16)
    wgT[:D] = np.ascontiguousarray(Wg.T).astype(BF16)
    wgT[D] = bg.astype(BF16)

    return xT, weT, wgT


def kernel(inputs, Wg, bg, We, be):
    from concourse.bass_utils import run_bass_kernel_spmd

    nc = _get_nc()
    xT, weT, wgT = _prep_host(inputs, Wg, bg, We, be)

    in_maps = []
    for c in range(NCORES):
        in_maps.append({
            "xT": np.ascontiguousarray(xT[:, c * TPC:(c + 1) * TPC]),
            "weT": weT,
            "wgT": wgT,
        })

    res = run_bass_kernel_spmd(nc, in_maps, core_ids=list(range(NCORES)))
    out = np.concatenate([r["out"] for r in res.results], axis=0)
    return out.reshape(B, S, H)


# revision 15
# speedup vs baseline: 1.1111x; 1.1111x over previous
"""Trainium2 Bass kernel for nn_MoELayer (dense MoE: gate softmax over 8
experts, all experts computed, gate-weighted sum).

Strategy: data-parallel over tokens. B*S = 8192 tokens are split across the
8 NeuronCores (1024 tokens each); every core holds all expert weights
(replicated) and computes its token slice end-to-end, so no collective is
needed and per-core outputs are disjoint slices of the final [B,S,H] output.

Device kernel (per core, SPMD):
  - x is staged host-side as x_T_aug [1152, 1024] bf16: rows 0..1023 are
    x.T (contraction dim on partitions), row 1024 is ones, rest zeros.
    Weights are staged as We_T_aug [8, 1152, 2048] with the bias be in row
    1024 — the bias add is folded into the matmul contraction (K = 9x128).
  - gate: logits via TensorE (N=8), softmax on DVE/ACT (reduce_max(negate),
    Exp(bias=-max), reduce_add, reciprocal, scale).
  - experts: for each expert, [128-token tile] x [512-wide H chunk] bf16
    matmuls accumulate K=9 chunks into a 4-bank PSUM tile [128, 2048].
    Epilogue fuses leaky-relu and gate weighting into ONE ScalarE op:
    Lrelu(g * x) = g * Lrelu(x) for g > 0 (softmax probs are positive),
    then a single VectorE add accumulates over experts.
"""

import numpy as np
import ml_dtypes

BF16 = ml_dtypes.bfloat16

B, S, D, H, E = 4, 2048, 1024, 2048, 8
NCORES = 8
TOK = B * S                 # 8192 tokens
TPC = TOK // NCORES         # 1024 tokens per core
P = 128
KCH = (D // P) + 1          # 9 contraction chunks (8 data + 1 bias/zeros)
KAUG = KCH * P              # 1152
NTT = TPC // P              # 8 token tiles per core
NHC = H // 512              # 4 H chunks

_CACHE = {}


def _build_nc(repeats=1, bias_via="matmul", epilogue="full",
              mm_order="hc_kc", mm_n=512, same_lhst=False):
    import concourse.mybir as mybir
    import concourse.tile as tile
    from concourse import bacc
    from concourse.bass import ts, ds

    fp32 = mybir.dt.float32
    bf16 = mybir.dt.bfloat16
    AF = mybir.ActivationFunctionType
    Alu = mybir.AluOpType

    nc = bacc.Bacc("TRN2", target_bir_lowering=False, debug=False)

    xT_d = nc.dram_tensor("xT", [KAUG, TPC], bf16, kind="ExternalInput")
    weT_d = nc.dram_tensor("weT", [E, KAUG, H], bf16, kind="ExternalInput")
    wgT_d = nc.dram_tensor("wgT", [KAUG, E], bf16, kind="ExternalInput")
    if bias_via == "dve":
        beR_d = nc.dram_tensor("beR", [E, P, H], fp32, kind="ExternalInput")
    out_d = nc.dram_tensor("out", [TPC, H], fp32, kind="ExternalOutput")
    KCH_E = KCH if bias_via == "matmul" else KCH - 1  # expert-matmul k chunks
    assert epilogue in ("full", "act", "none")

    with tile.TileContext(nc) as tc:
        with (
            tc.tile_pool(name="const", bufs=1) as const_pool,
            tc.tile_pool(name="wep", bufs=2) as we_pool,
            tc.tile_pool(name="accp", bufs=1) as acc_pool,
            tc.tile_pool(name="leakp", bufs=3) as leak_pool,
            tc.tile_pool(name="smallp", bufs=8) as small_pool,
        ):
            x_sb = const_pool.tile([P, KCH, TPC], bf16)
            nc.sync.dma_start(x_sb[:], xT_d.ap().rearrange("(c p) t -> p c t", p=P))
            wg_sb = const_pool.tile([P, KCH, E], bf16)
            nc.sync.dma_start(wg_sb[:], wgT_d.ap().rearrange("(c p) e -> p c e", p=P))

            g_all = const_pool.tile([P, NTT, E], fp32)
            acc = acc_pool.tile([P, NTT, H], fp32)
            if epilogue != "full":
                nc.vector.memset(acc, 0.0)

            # ---------------- gate phase ----------------
            with tc.tile_pool(name="gps", bufs=2, space="PSUM") as gps_pool:
                for tt in range(NTT):
                    gl = gps_pool.tile([P, E], fp32, tag="gl")
                    for kc in range(KCH):
                        nc.tensor.matmul(gl, x_sb[:, kc, ts(tt, P)],
                                         wg_sb[:, kc, :],
                                         start=(kc == 0), stop=(kc == KCH - 1))
                    negmax = small_pool.tile([P, 1], fp32, tag="negmax")
                    nc.vector.tensor_reduce(negmax, gl, axis=mybir.AxisListType.X,
                                            op=Alu.max, negate=True)
                    expd = small_pool.tile([P, E], fp32, tag="expd")
                    nc.scalar.activation(expd, gl, AF.Exp, bias=negmax, scale=1.0)
                    ssum = small_pool.tile([P, 1], fp32, tag="ssum")
                    nc.vector.tensor_reduce(ssum, expd, axis=mybir.AxisListType.X,
                                            op=Alu.add)
                    rec = small_pool.tile([P, 1], fp32, tag="rec")
                    nc.vector.reciprocal(rec, ssum)
                    nc.vector.tensor_scalar_mul(g_all[:, tt, :], expd, rec)

            # ---------------- expert phase ----------------
            with tc.tile_pool(name="mmps", bufs=2, space="PSUM") as mm_pool:
              for _rep in range(repeats):
                for e in range(E):
                    we_sb = we_pool.tile([P, KCH_E, H], bf16, tag="we")
                    nc.sync.dma_start(
                        we_sb[:],
                        weT_d.ap()[e, 0:KCH_E * P].rearrange("(c p) h -> p c h", p=P))
                    if bias_via == "dve":
                        be_sb = we_pool.tile([P, H], fp32, tag="be")
                        nc.sync.dma_start(be_sb[:], beR_d.ap()[e])
                    for tt in range(NTT):
                        ps = mm_pool.tile([P, H], fp32, tag="ps")
                        nhc_i = H // mm_n
                        if mm_order == "hc_kc":
                            for hc in range(nhc_i):
                                for kc in range(KCH_E):
                                    lhsT = (x_sb[:, 0, ts(0, P)] if same_lhst
                                            else x_sb[:, kc, ts(tt, P)])
                                    nc.tensor.matmul(
                                        ps[:, ds(hc * mm_n, mm_n)],
                                        lhsT,
                                        we_sb[:, kc, ds(hc * mm_n, mm_n)],
                                        start=(kc == 0), stop=(kc == KCH_E - 1))
                        else:  # kc outer: share lhsT across the hc chunks
                            for kc in range(KCH_E):
                                for hc in range(nhc_i):
                                    nc.tensor.matmul(
                                        ps[:, ds(hc * mm_n, mm_n)],
                                        x_sb[:, kc, ts(tt, P)],
                                        we_sb[:, kc, ds(hc * mm_n, mm_n)],
                                        start=(kc == 0), stop=(kc == KCH_E - 1),
                                        skip_group_check=True)
                        if epilogue == "none":
                            continue
                        if bias_via == "dve" and epilogue == "full":
                            nc.vector.tensor_add(ps, ps, be_sb)
                        gap = g_all[:, tt, ds(e, 1)]
                        if e == 0:
                            nc.scalar.activation(acc[:, tt, :], ps, AF.Lrelu,
                                                 scale=gap, alpha=0.01)
                        else:
                            leak = leak_pool.tile([P, H], fp32, tag="leak")
                            nc.scalar.activation(leak, ps, AF.Lrelu,
                                                 scale=gap, alpha=0.01)
                            if epilogue == "full":
                                nc.vector.tensor_add(acc[:, tt, :],
                                                     acc[:, tt, :], leak)

            for tt in range(NTT):
                nc.sync.dma_start(out_d.ap()[ts(tt, P), :], acc[:, tt, :])

    nc.compile()
    return nc


def _bias_via():
    import os
    return os.environ.get("KERNEL_BIAS_VIA", "matmul")


def _get_nc():
    if "nc" not in _CACHE:
        _CACHE["nc"] = _build_nc(bias_via=_bias_via())
    return _CACHE["nc"]


def _prep_host(inputs, Wg, bg, We, be):
    inputs = np.asarray(inputs, dtype=np.float32)
    Wg = np.asarray(Wg, dtype=np.float32)
    bg = np.asarray(bg, dtype=np.float32)
    We = np.asarray(We, dtype=np.float32)
    be = np.asarray(be, dtype=np.float32)

    xT = np.zeros((KAUG, TOK), BF16)
    xT[:D] = np.ascontiguousarray(inputs.reshape(TOK, D).T).astype(BF16)
    xT[D] = np.asarray(1.0, BF16)

    weT = np.zeros((E, KAUG, H), BF16)
    weT[:, :D] = np.ascontiguousarray(We.transpose(0, 2, 1)).astype(BF16)
    weT[:, D] = be.astype(BF16)

    wgT = np.zeros((KAUG, E), BF16)
    wgT[:D] = np.ascontiguousarray(Wg.T).astype(BF16)
    wgT[D] = bg.astype(BF16)

    return xT, weT, wgT


def kernel(inputs, Wg, bg, We, be):
    from concourse.bass_utils import run_bass_kernel_spmd

    nc = _get_nc()
    xT, weT, wgT = _prep_host(inputs, Wg, bg, We, be)

    in_maps = []
    for c in range(NCORES):
        m = {
            "xT": np.ascontiguousarray(xT[:, c * TPC:(c + 1) * TPC]),
            "weT": weT,
            "wgT": wgT,
        }
        if _bias_via() == "dve":
            m["beR"] = np.ascontiguousarray(np.broadcast_to(
                np.asarray(be, np.float32)[:, None, :], (E, P, H)))
        in_maps.append(m)

    res = run_bass_kernel_spmd(nc, in_maps, core_ids=list(range(NCORES)))
    out = np.concatenate([r["out"] for r in res.results], axis=0)
    return out.reshape(B, S, H)


# revision 22
# speedup vs baseline: 1.1201x; 1.0081x over previous
"""Trainium2 Bass kernel for nn_MoELayer (dense MoE: gate softmax over 8
experts, all experts computed, gate-weighted sum).

Strategy: data-parallel over tokens. B*S = 8192 tokens are split across the
8 NeuronCores (1024 tokens each); every core holds all expert weights
(replicated) and computes its token slice end-to-end, so no collective is
needed and per-core outputs are disjoint slices of the final [B,S,H] output.

Device kernel (per core, SPMD), HW-tuned via repeat-differencing bench:
  - x is staged host-side as x_T_aug [1152, 1024] bf16 (contraction dim on
    partitions; row 1024 is ones for the gate-bias matmul trick).
  - gate: logits via TensorE (N=8, K=9x128 incl. bias row), softmax on
    DVE/ACT (reduce_max(negate), Exp(bias=-max), reduce_add, reciprocal).
  - experts: per expert, [128-token tile] x [512-wide H chunk] bf16
    matmuls accumulate K=8x128 chunks into a 4-bank PSUM tile [128, 2048]
    (per-bank contiguous accumulation groups: hc outer / kc inner measured
    fastest; kc-outer and N=1024 variants were slower/illegal).
  - expert bias be is added on VectorE straight into PSUM (in-place
    tensor_add with an HBM-replicated [128, H] f32 bias tile) — measured
    ~90us/core faster than folding the bias as a 9th matmul K-chunk.
  - epilogue fuses leaky-relu and gate weighting into ONE ScalarE op:
    Lrelu(g * x) = g * Lrelu(x) for g > 0 (softmax probs are positive),
    then a single VectorE add accumulates over experts. The whole epilogue
    hides under the TensorE span.
"""

import numpy as np
import ml_dtypes

BF16 = ml_dtypes.bfloat16

B, S, D, H, E = 4, 2048, 1024, 2048, 8
NCORES = 8
TOK = B * S                 # 8192 tokens
TPC = TOK // NCORES         # 1024 tokens per core
P = 128
KCH = (D // P) + 1          # 9 contraction chunks (8 data + 1 bias/zeros)
KAUG = KCH * P              # 1152
NTT = TPC // P              # 8 token tiles per core
NHC = H // 512              # 4 H chunks

_CACHE = {}


def _build_nc(repeats=1, bias_via="dve", epilogue="full",
              mm_order="hc_kc", mm_n=512, same_lhst=False, ps_cols=512):
    import concourse.mybir as mybir
    import concourse.tile as tile
    from concourse import bacc
    from concourse.bass import ts, ds

    fp32 = mybir.dt.float32
    bf16 = mybir.dt.bfloat16
    AF = mybir.ActivationFunctionType
    Alu = mybir.AluOpType

    nc = bacc.Bacc("TRN2", target_bir_lowering=False, debug=False)

    xT_d = nc.dram_tensor("xT", [KAUG, TPC], bf16, kind="ExternalInput")
    weT_d = nc.dram_tensor("weT", [E, KAUG, H], bf16, kind="ExternalInput")
    wgT_d = nc.dram_tensor("wgT", [KAUG, E], bf16, kind="ExternalInput")
    if bias_via == "dve":
        beR_d = nc.dram_tensor("beR", [E, P, H], fp32, kind="ExternalInput")
    out_d = nc.dram_tensor("out", [TPC, H], fp32, kind="ExternalOutput")
    KCH_E = KCH if bias_via == "matmul" else KCH - 1  # expert-matmul k chunks
    assert epilogue in ("full", "act", "none")

    with tile.TileContext(nc) as tc:
        with (
            tc.tile_pool(name="const", bufs=1) as const_pool,
            tc.tile_pool(name="wep", bufs=2) as we_pool,
            tc.tile_pool(name="accp", bufs=1) as acc_pool,
            tc.tile_pool(name="leakp", bufs=8) as leak_pool,
            tc.tile_pool(name="smallp", bufs=8) as small_pool,
        ):
            x_sb = const_pool.tile([P, KCH, TPC], bf16)
            nc.sync.dma_start(x_sb[:], xT_d.ap().rearrange("(c p) t -> p c t", p=P))
            wg_sb = const_pool.tile([P, KCH, E], bf16)
            nc.sync.dma_start(wg_sb[:], wgT_d.ap().rearrange("(c p) e -> p c e", p=P))

            g_all = const_pool.tile([P, NTT, E], fp32)
            acc = acc_pool.tile([P, NTT, H], fp32)
            if epilogue != "full":
                nc.vector.memset(acc, 0.0)

            # ---------------- gate phase ----------------
            with tc.tile_pool(name="gps", bufs=2, space="PSUM") as gps_pool:
                for tt in range(NTT):
                    gl = gps_pool.tile([P, E], fp32, tag="gl")
                    for kc in range(KCH):
                        nc.tensor.matmul(gl, x_sb[:, kc, ts(tt, P)],
                                         wg_sb[:, kc, :],
                                         start=(kc == 0), stop=(kc == KCH - 1))
                    negmax = small_pool.tile([P, 1], fp32, tag="negmax")
                    nc.vector.tensor_reduce(negmax, gl, axis=mybir.AxisListType.X,
                                            op=Alu.max, negate=True)
                    expd = small_pool.tile([P, E], fp32, tag="expd")
                    nc.scalar.activation(expd, gl, AF.Exp, bias=negmax, scale=1.0)
                    ssum = small_pool.tile([P, 1], fp32, tag="ssum")
                    nc.vector.tensor_reduce(ssum, expd, axis=mybir.AxisListType.X,
                                            op=Alu.add)
                    rec = small_pool.tile([P, 1], fp32, tag="rec")
                    nc.vector.reciprocal(rec, ssum)
                    nc.vector.tensor_scalar_mul(g_all[:, tt, :], expd, rec)

            # ---------------- expert phase ----------------
            with tc.tile_pool(name="mmps", bufs=4096 // ps_cols,
                              space="PSUM") as mm_pool:
              for _rep in range(repeats):
                for e in range(E):
                    we_sb = we_pool.tile([P, KCH_E, H], bf16, tag="we")
                    nc.sync.dma_start(
                        we_sb[:],
                        weT_d.ap()[e, 0:KCH_E * P].rearrange("(c p) h -> p c h", p=P))
                    if bias_via == "dve":
                        be_sb = we_pool.tile([P, H], fp32, tag="be")
                        nc.sync.dma_start(be_sb[:], beR_d.ap()[e])
                    for tt in range(NTT):
                      for pst in range(H // ps_cols):
                        ps = mm_pool.tile([P, ps_cols], fp32, tag="ps")
                        po = pst * ps_cols
                        nhc_i = ps_cols // mm_n
                        if mm_order == "hc_kc":
                            for hc in range(nhc_i):
                                for kc in range(KCH_E):
                                    lhsT = (x_sb[:, 0, ts(0, P)] if same_lhst
                                            else x_sb[:, kc, ts(tt, P)])
                                    nc.tensor.matmul(
                                        ps[:, ds(hc * mm_n, mm_n)],
                                        lhsT,
                                        we_sb[:, kc, ds(po + hc * mm_n, mm_n)],
                                        start=(kc == 0), stop=(kc == KCH_E - 1))
                        else:  # kc outer: share lhsT across the hc chunks
                            for kc in range(KCH_E):
                                for hc in range(nhc_i):
                                    nc.tensor.matmul(
                                        ps[:, ds(hc * mm_n, mm_n)],
                                        x_sb[:, kc, ts(tt, P)],
                                        we_sb[:, kc, ds(po + hc * mm_n, mm_n)],
                                        start=(kc == 0), stop=(kc == KCH_E - 1),
                                        skip_group_check=True)
                        if epilogue == "none":
                            continue
                        if bias_via == "dve" and epilogue == "full":
                            nc.vector.tensor_add(ps, ps, be_sb[:, ds(po, ps_cols)])
                        gap = g_all[:, tt, ds(e, 1)]
                        if e == 0:
                            nc.scalar.activation(acc[:, tt, ds(po, ps_cols)], ps,
                                                 AF.Lrelu, scale=gap, alpha=0.01)
                        else:
                            leak = leak_pool.tile([P, ps_cols], fp32, tag="leak")
                            nc.scalar.activation(leak, ps, AF.Lrelu,
                                                 scale=gap, alpha=0.01)
                            if epilogue == "full":
                                nc.vector.tensor_add(
                                    acc[:, tt, ds(po, ps_cols)],
                                    acc[:, tt, ds(po, ps_cols)], leak)

            for tt in range(NTT):
                nc.sync.dma_start(out_d.ap()[ts(tt, P), :], acc[:, tt, :])

    nc.compile()
    return nc


def _bias_via():
    import os
    return os.environ.get("KERNEL_BIAS_VIA", "dve")


def _get_nc():
    if "nc" not in _CACHE:
        _CACHE["nc"] = _build_nc(bias_via=_bias_via())
    return _CACHE["nc"]


def _prep_host(inputs, Wg, bg, We, be):
    inputs = np.asarray(inputs, dtype=np.float32)
    Wg = np.asarray(Wg, dtype=np.float32)
    bg = np.asarray(bg, dtype=np.float32)
    We = np.asarray(We, dtype=np.float32)
    be = np.asarray(be, dtype=np.float32)

    xT = np.zeros((KAUG, TOK), BF16)
    xT[:D] = np.ascontiguousarray(inputs.reshape(TOK, D).T).astype(BF16)
    xT[D] = np.asarray(1.0, BF16)

    weT = np.zeros((E, KAUG, H), BF16)
    weT[:, :D] = np.ascontiguousarray(We.transpose(0, 2, 1)).astype(BF16)
    weT[:, D] = be.astype(BF16)

    wgT = np.zeros((KAUG, E), BF16)
    wgT[:D] = np.ascontiguousarray(Wg.T).astype(BF16)
    wgT[D] = bg.astype(BF16)

    return xT, weT, wgT


def kernel(inputs, Wg, bg, We, be):
    from concourse.bass_utils import run_bass_kernel_spmd

    nc = _get_nc()
    xT, weT, wgT = _prep_host(inputs, Wg, bg, We, be)

    in_maps = []
    for c in range(NCORES):
        m = {
            "xT": np.ascontiguousarray(xT[:, c * TPC:(c + 1) * TPC]),
            "weT": weT,
            "wgT": wgT,
        }
        if _bias_via() == "dve":
            m["beR"] = np.ascontiguousarray(np.broadcast_to(
                np.asarray(be, np.float32)[:, None, :], (E, P, H)))
        in_maps.append(m)

    res = run_bass_kernel_spmd(nc, in_maps, core_ids=list(range(NCORES)))
    out = np.concatenate([r["out"] for r in res.results], axis=0)
    return out.reshape(B, S, H)
